# revision 1
# baseline (speedup 1.0000x reference)
"""GNN message-passing net on 8 Trainium2 cores.

Reference: x:[256,784,1] -> h1 = elu(spmm(x)@W1+b1) -> h2 = elu(spmm(h1)@W2+b2)
-> flat[B, N*C] -> relu(flat@Wf1+bf1) -> softmax(z@Wf2+bf2).

Strategy (all matmul operands bf16, fp32 PSUM accumulation):
  * Densify the sparse filter A (784x784, ~1% nz) on the host; spmm becomes
    dense matmuls on the PE array.
  * F=1 makes conv1 an outer product: out1 = A @ X^T [784,256] shared by all
    channels; h1_c = elu(W1[c]*out1+b1[c]) via ACT Exp/Relu with fused
    per-partition scale/bias + 2 DVE ops (elu(t)=min(exp(t),1)+relu(t)-1).
  * Conv2 spmm channel-sharded: core k computes out2_c = A @ h1_c for
    channels 4k..4k+3, full batch (free=256).
  * AllToAll reshards channel->node: core j receives all 32 pre-mix channels
    for nodes [112j, 112j+112) (core 7 gets zero-padded nodes 784..895),
    packed [(ng,c) partitions, s, b] with its nodes split 4x28.
  * W2 channel mix as a 128x128 stationary kron(I4,W2) matmul; +b2, elu.
  * FC1 stays K-sharded: core k holds Wf1 rows for its nodes (zero rows for
    pad nodes), 28 K-chunks x 4 h-chunks, free=256.  z^T partials [512,256]
    are ReduceScattered; each core then does +bf1, relu, FC2 (+bf2 via a
    ones-row matmul) and softmax for its 32-batch block.
"""
import json

import numpy as np

import concourse.bass as bass
import concourse.mybir as mybir
import concourse.tile as tile
from concourse.bass_utils import run_bass_kernel_spmd

B, N, F, E = 256, 784, 1, 6272
C, H, N_OUT = 32, 512, 10
NCORE = 8
CPC = C // NCORE      # 4 channels per core in conv2
P = 112               # 784 = 7 * 112
KN = N // P           # 7 node chunks
NPAD = P * NCORE      # 896 padded nodes for the node reshard
NG = 4                # node groups packed into partitions for the mix
NS = P // NG          # 28 nodes per group per core
BPC = B // NCORE      # 32 batch rows per core
HJ = H // 128         # 4 h chunks

f32 = mybir.dt.float32
bf16 = mybir.dt.bfloat16
AF = mybir.ActivationFunctionType
ALU = mybir.AluOpType
AX = mybir.AxisListType


# ---------------------------------------------------------------------------
# BIR post-pass: this walrus build rejects instructions with >1 sync-wait;
# split extras onto standalone EventSemaphore instructions (same engine,
# inserted just before, so the engine stream stalls identically).
def _split_waits(bir: dict, max_waits: int = 1) -> dict:
    n = [0]
    for fn in bir.get("functions", []):
        for blk in fn.get("blocks", []):
            out = []
            for ins in blk.get("instructions", []):
                si = ins.get("sync_info") or {}
                waits = si.get("on_wait") or []
                if len(waits) > max_waits:
                    for w in waits[max_waits:]:
                        n[0] += 1
                        out.append({
                            "name": f"I-waitsplit-{n[0]}",
                            "opcode": "EventSemaphore",
                            "engine": ins["engine"],
                            "ins": [], "outs": [],
                            **({"debug": ins["debug"]} if "debug" in ins else {}),
                            "sync_info": {"on_update": [], "on_wait": [w]},
                        })
                    si = dict(si)
                    si["on_wait"] = waits[:max_waits]
                    ins = dict(ins)
                    ins["sync_info"] = si
                out.append(ins)
            blk["instructions"] = out
    return bir


def _install_wait_splitter(nc):
    orig = nc.to_json_bytes
    nc.to_json_bytes = lambda: json.dumps(_split_waits(json.loads(orig()))).encode()


# ---------------------------------------------------------------------------
def _build_program():
    nc = bass.Bass(num_devices=NCORE)

    at_d = nc.dram_tensor("at", [P, KN * N], bf16, kind="ExternalInput")
    xt_d = nc.dram_tensor("xt", [P, KN * B], bf16, kind="ExternalInput")
    wf1_d = nc.dram_tensor("wf1", [NS * 128, H], bf16, kind="ExternalInput")
    wb_d = nc.dram_tensor("wb", [1, 2 * CPC], f32, kind="ExternalInput")
    w2k_d = nc.dram_tensor("w2k", [128, 128], bf16, kind="ExternalInput")
    b2k_d = nc.dram_tensor("b2k", [128, 1], f32, kind="ExternalInput")
    bf1_d = nc.dram_tensor("bf1", [128, HJ], f32, kind="ExternalInput")
    wf2_d = nc.dram_tensor("wf2", [128, HJ * N_OUT], bf16, kind="ExternalInput")
    bf2_d = nc.dram_tensor("bf2", [1, N_OUT], bf16, kind="ExternalInput")
    out_d = nc.dram_tensor("out", [BPC, N_OUT], f32, kind="ExternalOutput")

    with tile.TileContext(nc) as tc:
        with (
            tc.tile_pool(name="big", bufs=1) as big,
            tc.tile_pool(name="small", bufs=1) as small,
            tc.tile_pool(name="ework", bufs=4) as ework,
            tc.tile_pool(name="rwork", bufs=4) as rwork,
            tc.tile_pool(name="ps_a", bufs=2, space="PSUM") as ps_a,
            tc.tile_pool(name="ps_b", bufs=2, space="PSUM") as ps_b,
            tc.tile_pool(name="ps_fc1", bufs=4, space="PSUM") as ps_fc1,
            tc.tile_pool(name="dram", bufs=1, space="DRAM") as dram,
        ):
            # ---- resident inputs ------------------------------------------
            at_sb = big.tile([P, KN, N], bf16)
            xt_sb = big.tile([P, KN, B], bf16)
            wf1_sb = big.tile([128, NS, H], bf16)
            wb_sb = small.tile([1, 2 * CPC], f32)
            w2k_sb = small.tile([128, 128], bf16)
            b2k_sb = small.tile([128, 1], f32)
            bf1_sb = small.tile([128, HJ], f32)
            wf2_sb = small.tile([128, HJ, N_OUT], bf16)
            bf2_sb = small.tile([1, N_OUT], bf16)
            ones = small.tile([1, 128], bf16)
            ones_f = small.tile([1, 128], f32)

            xt_ap = xt_d[:].rearrange("p (k b) -> p k b", k=KN)
            at_ap = at_d[:].rearrange("p (k n) -> p k n", k=KN)
            for kc in range(KN):
                nc.sync.dma_start(xt_sb[:, kc, :], xt_ap[:, kc, :])
                nc.sync.dma_start(at_sb[:, kc, :], at_ap[:, kc, :])
            nc.sync.dma_start(wb_sb[:], wb_d[:])
            nc.sync.dma_start(w2k_sb[:], w2k_d[:])
            nc.sync.dma_start(b2k_sb[:], b2k_d[:])
            nc.sync.dma_start(bf1_sb[:], bf1_d[:])
            nc.sync.dma_start(wf2_sb[:], wf2_d[:].rearrange("p (j o) -> p j o", j=HJ))
            nc.sync.dma_start(bf2_sb[:], bf2_d[:])
            nc.sync.dma_start(wf1_sb[:],
                              wf1_d[:].rearrange("(s p) h -> p s h", p=128))
            nc.vector.memset(ones[:], 1.0)
            nc.vector.memset(ones_f[:], 1.0)

            # ---- broadcast W1/b1 channel scalars across partitions --------
            ps_bc = ps_a.tile([128, 512], f32, tag="psa")
            nc.tensor.matmul(ps_bc[:, 0:2 * CPC], ones_f[0:1, 0:128],
                             wb_sb[:])
            wband = small.tile([128, 2 * CPC], f32)
            nc.vector.tensor_copy(wband[:], ps_bc[:, 0:2 * CPC])

            # ---- conv1 + elu ----------------------------------------------
            h1_sb = big.tile([P, CPC, KN, B], bf16)
            for mc in range(KN):
                o1 = ps_a.tile([128, 512], f32, tag="psa")
                for kc in range(KN):
                    nc.tensor.matmul(
                        o1[0:P, 0:B],
                        at_sb[:, kc, mc * P:(mc + 1) * P],
                        xt_sb[:, kc, :],
                        start=(kc == 0), stop=(kc == KN - 1),
                    )
                for c in range(CPC):
                    sc = wband[0:P, c:c + 1]
                    bi = wband[0:P, CPC + c:CPC + c + 1]
                    e = ework.tile([128, 512], f32)
                    nc.scalar.activation(e[0:P, 0:B], o1[0:P, 0:B], AF.Exp,
                                         bias=bi, scale=sc)
                    r = rwork.tile([128, 512], f32)
                    nc.scalar.activation(r[0:P, 0:B], o1[0:P, 0:B], AF.Relu,
                                         bias=bi, scale=sc)
                    nc.vector.tensor_scalar(e[0:P, 0:B], e[0:P, 0:B],
                                            1.0, -1.0, ALU.min, ALU.add)
                    nc.vector.tensor_tensor(h1_sb[:, c, mc, :], e[0:P, 0:B],
                                            r[0:P, 0:B], ALU.add)

            # ---- AllToAll: channel shard -> node shard --------------------
            # block j = (4 local channels, nodes [112j,112j+112), all b);
            # node block 7 (nodes 784..895) is zero padding.
            r_sb = big.tile([128, NS, B], bf16)
            a2a_in = dram.tile([NCORE, CPC * P, B], bf16)
            a2a_out = dram.tile([NCORE, CPC * P, B], bf16)
            zpad = small.tile([P, CPC, B], bf16)
            nc.vector.memset(zpad[:], 0.0)

            def a2a_all():
                for j in range(NCORE):
                    dst = a2a_in[j].rearrange("(cl p) b -> p cl b", cl=CPC)
                    if j < KN:
                        nc.gpsimd.dma_start(dst, out2_sb[:, :, j, :])
                    else:
                        nc.gpsimd.dma_start(dst, zpad[:])
                nc.gpsimd.collective_compute(
                    "AllToAll", ALU.bypass,
                    replica_groups=[list(range(NCORE))],
                    ins=[a2a_in.opt()], outs=[a2a_out.opt()],
                )
                ap = a2a_out[:].rearrange("k (cl g s) b -> g (k cl) s b",
                                          cl=CPC, g=NG)
                for ng in range(NG):
                    nc.gpsimd.dma_start(r_sb[ng * C:(ng + 1) * C, :, :],
                                        ap[ng])

            # ---- conv2 spmm (channel shard) -> out2_sb (bf16) -------------
            out2_sb = big.tile([P, CPC, KN, B], bf16)
            for cp in range(0, CPC, 2):
                for mo in range(KN):
                    o2 = ps_b.tile([P, 2, B], f32, tag="psb")
                    for kc in range(KN):
                        nc.tensor.matmul(
                            o2[:],
                            at_sb[:, kc, mo * P:(mo + 1) * P],
                            h1_sb[:, cp:cp + 2, kc, :],
                            start=(kc == 0), stop=(kc == KN - 1),
                        )
                    if mo % 2 == 0:
                        nc.vector.tensor_copy(out2_sb[:, cp:cp + 2, mo, :], o2[:])
                    else:
                        nc.scalar.copy(out2_sb[:, cp:cp + 2, mo, :], o2[:])
            a2a_all()

            # ---- W2 mix + b2 + elu -> h2 (bf16) ---------------------------
            h2_sb = big.tile([128, NS, B], bf16)
            for s0 in range(0, NS, 2):
                sw = min(2, NS - s0)
                fw = sw * B
                pm = ps_a.tile([128, 512], f32, tag="psa")
                nc.tensor.matmul(pm[:, 0:fw], w2k_sb[:],
                                 r_sb[:, s0:s0 + sw, :])
                e = ework.tile([128, 512], f32)
                nc.scalar.activation(e[:, 0:fw], pm[:, 0:fw], AF.Exp,
                                     bias=b2k_sb[:, 0:1])
                r = rwork.tile([128, 512], f32)
                nc.scalar.activation(r[:, 0:fw], pm[:, 0:fw], AF.Relu,
                                     bias=b2k_sb[:, 0:1])
                nc.vector.tensor_scalar(e[:, 0:fw], e[:, 0:fw],
                                        1.0, -1.0, ALU.min, ALU.add)
                nc.vector.tensor_tensor(h2_sb[:, s0:s0 + sw, :], e[:, 0:fw],
                                        r[:, 0:fw], ALU.add)

            # ---- FC1 (K shard): z^T partials [512, 256] -------------------
            zps = [ps_fc1.tile([128, B], f32, tag="psfc1", name=f"zp{h}")
                   for h in range(HJ)]
            for s in range(NS):
                for hj in range(HJ):
                    nc.tensor.matmul(
                        zps[hj][:],
                        wf1_sb[:, s, hj * 128:(hj + 1) * 128],
                        h2_sb[:, s, :],
                        start=(s == 0), stop=(s == NS - 1),
                    )

            # ---- ReduceScatter z partials ---------------------------------
            zsb = small.tile([128, HJ, B], f32)
            for hj in range(HJ):
                nc.scalar.copy(zsb[:, hj, :], zps[hj][:])
            rs_in = dram.tile([NCORE, H, BPC], f32)
            rs_out = dram.tile([H, BPC], f32)
            rdst = rs_in[:].rearrange("j h b -> h j b")
            for hj in range(HJ):
                nc.gpsimd.dma_start(
                    rdst[hj * 128:(hj + 1) * 128],
                    zsb[:, hj, :].rearrange("h (j b) -> h j b", j=NCORE),
                )
            nc.gpsimd.collective_compute(
                "ReduceScatter", ALU.add,
                replica_groups=[list(range(NCORE))],
                ins=[rs_in.opt()], outs=[rs_out.opt()],
            )

            # ---- +bf1, relu, FC2, +bf2, softmax ---------------------------
            z_sb = small.tile([128, HJ, BPC], f32)
            zr_sb = small.tile([128, HJ, BPC], bf16)
            for hj in range(HJ):
                nc.sync.dma_start(z_sb[:, hj, :],
                                  rs_out[hj * 128:(hj + 1) * 128, :])
                nc.scalar.activation(zr_sb[:, hj, :], z_sb[:, hj, :], AF.Relu,
                                     bias=bf1_sb[:, hj:hj + 1])

            ps_o = ps_b.tile([BPC, N_OUT], f32, tag="psb")
            for hj in range(HJ):
                nc.tensor.matmul(ps_o[:], zr_sb[:, hj, :], wf2_sb[:, hj, :],
                                 start=(hj == 0), stop=False)
            nc.tensor.matmul(ps_o[:], ones[0:1, 0:BPC], bf2_sb[:],
                             start=False, stop=True)

            mx = small.tile([BPC, 1], f32)
            nc.vector.tensor_reduce(mx[:], ps_o[:], axis=AX.X, op=ALU.max,
                                    negate=True)
            t = small.tile([BPC, N_OUT], f32)
            nc.vector.tensor_scalar(t[:], ps_o[:], mx[0:BPC, 0:1], None, ALU.add)
            ex = small.tile([BPC, N_OUT], f32)
            nc.scalar.activation(ex[:], t[:], AF.Exp)
            sm = small.tile([BPC, 1], f32)
            nc.vector.tensor_reduce(sm[:], ex[:], axis=AX.X, op=ALU.add)
            rc = small.tile([BPC, 1], f32)
            nc.vector.reciprocal(rc[:], sm[:])
            ob = small.tile([BPC, N_OUT], f32)
            nc.vector.tensor_scalar(ob[:], ex[:], rc[0:BPC, 0:1], None, ALU.mult)
            nc.sync.dma_start(out_d[:], ob[:])

    _install_wait_splitter(nc)
    return nc


_NC_CACHE = None


def _get_program():
    global _NC_CACHE
    if _NC_CACHE is None:
        _NC_CACHE = _build_program()
    return _NC_CACHE


# ---------------------------------------------------------------------------
def _prep_inputs(x, edge_row, edge_col, edge_val, W1, b1, W2, b2,
                 Wf1, bf1, Wf2, bf2):
    import ml_dtypes
    f = np.float32
    bf = ml_dtypes.bfloat16
    A = np.zeros((N, N), f)
    np.add.at(A, (np.asarray(edge_row), np.asarray(edge_col)),
              np.asarray(edge_val, f))
    AT = np.ascontiguousarray(A.T)                                  # [m, n]
    at = np.ascontiguousarray(
        AT.reshape(KN, P, N).transpose(1, 0, 2).reshape(P, KN * N)).astype(bf)

    XT = np.ascontiguousarray(np.asarray(x, f)[:, :, 0].T)          # [N, B]
    xt = np.ascontiguousarray(
        XT.reshape(KN, P, B).transpose(1, 0, 2).reshape(P, KN * B)).astype(bf)

    W1 = np.asarray(W1, f); b1 = np.asarray(b1, f)
    W2 = np.asarray(W2, f); b2 = np.asarray(b2, f)
    Wf1 = np.asarray(Wf1, f); bf1 = np.asarray(bf1, f)
    Wf2 = np.asarray(Wf2, f); bf2 = np.asarray(bf2, f)

    # mix weight: lhsT[(ng,c),(ng',c')] = delta(ng,ng') * W2[c,c']
    w2k = np.kron(np.eye(NG, dtype=f), W2).astype(bf)               # [128,128]
    b2k = np.tile(b2, NG).reshape(128, 1).astype(f)

    # FC1: core k's K-chunk s holds flat rows (n=112k+ng*28+s)*C + c' at
    # partition p = ng*C + c'; rows for pad nodes (n >= 784) are zero.
    Wf1_pad = np.zeros((NPAD, C, H), f)
    Wf1_pad[:N] = Wf1.reshape(N, C, H)

    bf1_l = np.ascontiguousarray(bf1.reshape(HJ, 128).T)            # [128, HJ]
    wf2_l = np.ascontiguousarray(
        Wf2.reshape(HJ, 128, N_OUT).transpose(1, 0, 2).reshape(
            128, HJ * N_OUT)).astype(bf)
    bf2_l = bf2.reshape(1, N_OUT).astype(bf)

    in_maps = []
    for k in range(NCORE):
        wb = np.concatenate([W1[0, k * CPC:(k + 1) * CPC],
                             b1[k * CPC:(k + 1) * CPC]]).reshape(1, 2 * CPC)
        # [NG, NS, C, H] -> chunk s, partition (ng, c')
        wk = Wf1_pad[k * P:(k + 1) * P].reshape(NG, NS, C, H)
        wf1_l = np.ascontiguousarray(
            wk.transpose(1, 0, 2, 3).reshape(NS * 128, H)).astype(bf)
        in_maps.append({
            "at": at, "xt": xt, "wf1": wf1_l,
            "wb": np.ascontiguousarray(wb.astype(f)),
            "w2k": w2k, "b2k": b2k,
            "bf1": bf1_l, "wf2": wf2_l, "bf2": bf2_l,
        })
    return in_maps


def kernel(x, edge_row, edge_col, edge_val, W1, b1, W2, b2,
           Wf1, bf1, Wf2, bf2, **kw):
    nc = _get_program()
    in_maps = _prep_inputs(x, edge_row, edge_col, edge_val, W1, b1, W2, b2,
                           Wf1, bf1, Wf2, bf2)
    res = run_bass_kernel_spmd(nc, in_maps, list(range(NCORE)), **kw)
    out = np.concatenate([res.results[k]["out"] for k in range(NCORE)], axis=0)
    if kw.get("trace"):
        kernel.last_exec_time_ns = res.exec_time_ns
    return out.astype(np.float32)



# revision 4
# speedup vs baseline: 1.0072x; 1.0072x over previous
"""GNN message-passing net on 8 Trainium2 cores.

Reference: x:[256,784,1] -> h1 = elu(spmm(x)@W1+b1) -> h2 = elu(spmm(h1)@W2+b2)
-> flat[B, N*C] -> relu(flat@Wf1+bf1) -> softmax(z@Wf2+bf2).

Strategy (all matmul operands bf16, fp32 PSUM accumulation):
  * Densify the sparse filter A (784x784, ~1% nz) on the host; spmm becomes
    dense matmuls on the PE array.
  * F=1 makes conv1 an outer product: out1 = A @ X^T [784,256] shared by all
    channels; h1_c = elu(W1[c]*out1+b1[c]) with per-channel big-tile elu
    (elu(t)=min(exp(t),1)+relu(t)-1) over o1 staged in SBUF.
  * Conv2 spmm channel-sharded: core k computes out2_c = A @ h1_c for
    channels 4k..4k+3, full batch.  The channel->node AllToAll is split in
    two channel-pair halves so transfer overlaps conv2 / mix compute.
  * W2 mix runs as two K=64 matmul passes (one per A2A half); the first
    pass is staged to SBUF so it can run during the second transfer.
  * FC1 K-sharded with z in [batch, h] layout (F=512 matmuls); each core
    adds bf1/8 before a bf16 ReduceScatter over batch blocks.
  * Tail: relu, PE-transpose of z, FC2 (+bf2 via ones-row matmul), softmax
    without max-subtraction (logits are O(4)), Exp with fused row-sum.
"""
import json

import numpy as np

import concourse.bass as bass
import concourse.mybir as mybir
import concourse.tile as tile
from concourse.bass_utils import run_bass_kernel_spmd

B, N, F, E = 256, 784, 1, 6272
C, H, N_OUT = 32, 512, 10
NCORE = 8
CPC = C // NCORE      # 4 channels per core in conv2
P = 112               # 784 = 7 * 112
KN = N // P           # 7 node chunks
NPAD = P * NCORE      # 896 padded nodes for the node reshard
NG = 4                # node groups packed into partitions for the mix
NS = P // NG          # 28 nodes per group per core
BPC = B // NCORE      # 32 batch rows per core
HJ = H // 128         # 4 h chunks

f32 = mybir.dt.float32
bf16 = mybir.dt.bfloat16
AF = mybir.ActivationFunctionType
ALU = mybir.AluOpType
AX = mybir.AxisListType


# ---------------------------------------------------------------------------
# BIR post-pass: this walrus build rejects instructions with >1 sync-wait;
# split extras onto standalone EventSemaphore instructions (same engine,
# inserted just before, so the engine stream stalls identically).
def _split_waits(bir: dict, max_waits: int = 1) -> dict:
    n = [0]
    for fn in bir.get("functions", []):
        for blk in fn.get("blocks", []):
            out = []
            for ins in blk.get("instructions", []):
                si = ins.get("sync_info") or {}
                waits = si.get("on_wait") or []
                if len(waits) > max_waits:
                    for w in waits[max_waits:]:
                        n[0] += 1
                        out.append({
                            "name": f"I-waitsplit-{n[0]}",
                            "opcode": "EventSemaphore",
                            "engine": ins["engine"],
                            "ins": [], "outs": [],
                            **({"debug": ins["debug"]} if "debug" in ins else {}),
                            "sync_info": {"on_update": [], "on_wait": [w]},
                        })
                    si = dict(si)
                    si["on_wait"] = waits[:max_waits]
                    ins = dict(ins)
                    ins["sync_info"] = si
                out.append(ins)
            blk["instructions"] = out
    return bir


def _install_wait_splitter(nc):
    orig = nc.to_json_bytes
    nc.to_json_bytes = lambda: json.dumps(_split_waits(json.loads(orig()))).encode()


# ---------------------------------------------------------------------------
def _build_program():
    nc = bass.Bass(num_devices=NCORE)

    at_d = nc.dram_tensor("at", [P, KN * N], bf16, kind="ExternalInput")
    xt_d = nc.dram_tensor("xt", [P, KN * B], bf16, kind="ExternalInput")
    wf1_d = nc.dram_tensor("wf1", [NS * 128, H], bf16, kind="ExternalInput")
    wb_d = nc.dram_tensor("wb", [1, 2 * CPC], f32, kind="ExternalInput")
    w2k0_d = nc.dram_tensor("w2k0", [64, 128], bf16, kind="ExternalInput")
    w2k1_d = nc.dram_tensor("w2k1", [64, 128], bf16, kind="ExternalInput")
    b2k_d = nc.dram_tensor("b2k", [128, 1], f32, kind="ExternalInput")
    bf18_d = nc.dram_tensor("bf18", [1, H], f32, kind="ExternalInput")
    wf2_d = nc.dram_tensor("wf2", [128, HJ * N_OUT], bf16, kind="ExternalInput")
    bf2_d = nc.dram_tensor("bf2", [1, N_OUT], bf16, kind="ExternalInput")
    id_d = nc.dram_tensor("idm", [BPC, BPC], f32, kind="ExternalInput")
    out_d = nc.dram_tensor("out", [BPC, N_OUT], f32, kind="ExternalOutput")

    with tile.TileContext(nc) as tc:
        with (
            tc.tile_pool(name="big", bufs=1) as big,
            tc.tile_pool(name="small", bufs=1) as small,
            tc.tile_pool(name="ework", bufs=3) as ework,
            tc.tile_pool(name="rwork", bufs=3) as rwork,
            tc.tile_pool(name="ps1", bufs=2, space="PSUM") as ps1,
            tc.tile_pool(name="ps2", bufs=2, space="PSUM") as ps2,
            tc.tile_pool(name="psm", bufs=2, space="PSUM") as psm,
            tc.tile_pool(name="psf", bufs=2, space="PSUM") as psf,
            tc.tile_pool(name="dram", bufs=1, space="DRAM") as dram,
        ):
            # ---- resident inputs ------------------------------------------
            at_sb = big.tile([P, KN, N], bf16)
            xt_sb = big.tile([P, KN, B], bf16)
            wf1_sb = big.tile([128, NS, H], bf16)
            wb_sb = small.tile([1, 2 * CPC], f32)
            w2k0_sb = small.tile([64, 128], bf16)
            w2k1_sb = small.tile([64, 128], bf16)
            b2k_sb = small.tile([128, 1], f32)
            bf18_sb = small.tile([1, H], f32)
            wf2_sb = small.tile([128, HJ, N_OUT], bf16)
            bf2_sb = small.tile([1, N_OUT], bf16)
            id_sb = small.tile([BPC, BPC], f32)
            ones = small.tile([1, 128], bf16)
            ones_f = small.tile([1, 128], f32)

            xt_ap = xt_d[:].rearrange("p (k b) -> p k b", k=KN)
            at_ap = at_d[:].rearrange("p (k n) -> p k n", k=KN)
            # sync queue order = availability order for conv1 startup
            nc.sync.dma_start(xt_sb[:, 0:4, :], xt_ap[:, 0:4, :])
            nc.sync.dma_start(at_sb[:, 0:4, :], at_ap[:, 0:4, :])
            nc.sync.dma_start(wb_sb[:], wb_d[:])
            nc.sync.dma_start(xt_sb[:, 4:KN, :], xt_ap[:, 4:KN, :])
            nc.sync.dma_start(at_sb[:, 4:KN, :], at_ap[:, 4:KN, :])
            nc.sync.dma_start(w2k0_sb[:], w2k0_d[:])
            nc.sync.dma_start(w2k1_sb[:], w2k1_d[:])
            nc.sync.dma_start(b2k_sb[:], b2k_d[:])
            nc.sync.dma_start(bf18_sb[:], bf18_d[:])
            nc.sync.dma_start(wf2_sb[:], wf2_d[:].rearrange("p (j o) -> p j o", j=HJ))
            nc.sync.dma_start(bf2_sb[:], bf2_d[:])
            nc.sync.dma_start(id_sb[:], id_d[:])
            nc.sync.dma_start(wf1_sb[:],
                              wf1_d[:].rearrange("(s p) h -> p s h", p=128))
            nc.vector.memset(ones[:], 1.0)
            nc.vector.memset(ones_f[:], 1.0)

            # ---- broadcast W1/b1 channel scalars across partitions --------
            ps_bc = ps1.tile([128, 2 * CPC], f32, tag="ps1")
            nc.tensor.matmul(ps_bc[:], ones_f[0:1, 0:128], wb_sb[:])
            wband = small.tile([128, 2 * CPC], f32)
            nc.vector.tensor_copy(wband[:], ps_bc[:])

            # ---- conv1: o1 = A @ X^T staged to SBUF -----------------------
            o1s = big.tile([P, KN, B], f32)
            for mc in range(KN):
                o1 = ps1.tile([P, B], f32, tag="ps1")
                for kc in range(KN):
                    nc.tensor.matmul(
                        o1[:],
                        at_sb[:, kc, mc * P:(mc + 1) * P],
                        xt_sb[:, kc, :],
                        start=(kc == 0), stop=(kc == KN - 1),
                    )
                if mc % 2 == 0:
                    nc.scalar.copy(o1s[:, mc, :], o1[:])
                else:
                    nc.vector.tensor_copy(o1s[:, mc, :], o1[:])

            # bf1/8 broadcast across partitions (needed only at fc1 time)
            ps_bf = psm.tile([128, H], f32, tag="psm")
            nc.tensor.matmul(ps_bf[:], ones_f[0:1, 0:128], bf18_sb[:])
            bf1b = small.tile([128, H], f32)
            nc.vector.tensor_copy(bf1b[:], ps_bf[:])

            # ---- conv1 elu: h1_c = elu(w1_c * o1 + b1_c), big tiles -------
            h1_sb = big.tile([P, CPC, KN, B], bf16)
            for c, k0, k1 in ((0, 0, 4), (1, 0, 4), (0, 4, KN), (1, 4, KN),
                              (2, 0, 4), (3, 0, 4), (2, 4, KN), (3, 4, KN)):
                fw = (k1 - k0) * B
                sc = wband[0:P, c:c + 1]
                bi = wband[0:P, CPC + c:CPC + c + 1]
                e = ework.tile([P, 4 * B], f32)
                nc.scalar.activation(e[:, 0:fw], o1s[:, k0:k1, :], AF.Exp,
                                     bias=bi, scale=sc)
                r = rwork.tile([P, 4 * B], f32)
                nc.scalar.activation(r[:, 0:fw], o1s[:, k0:k1, :], AF.Relu,
                                     bias=bi, scale=sc)
                nc.vector.tensor_scalar(e[:, 0:fw], e[:, 0:fw],
                                        1.0, -1.0, ALU.min, ALU.add)
                nc.vector.tensor_tensor(h1_sb[:, c, k0:k1, :], e[:, 0:fw],
                                        r[:, 0:fw], ALU.add)

            # ---- conv2 spmm + split AllToAll (channel-pair halves) --------
            out2_sb = big.tile([P, CPC, KN, B], bf16)
            zpad = small.tile([P, 2, B], bf16)
            nc.vector.memset(zpad[:], 0.0)
            a2a_in = [dram.tile([NCORE, 2 * P, B], bf16, name=f"a2ai{i}")
                      for i in range(2)]
            a2a_out = [dram.tile([NCORE, 2 * P, B], bf16, name=f"a2ao{i}")
                       for i in range(2)]
            r_sb = [big.tile([64, NS, B], bf16, name=f"rsb{i}")
                    for i in range(2)]

            for cpi, cp in enumerate((0, 2)):
                for mo in range(KN):
                    o2 = ps2.tile([P, 2, B], f32, tag="ps2")
                    for kc in range(KN):
                        nc.tensor.matmul(
                            o2[:],
                            at_sb[:, kc, mo * P:(mo + 1) * P],
                            h1_sb[:, cp:cp + 2, kc, :],
                            start=(kc == 0), stop=(kc == KN - 1),
                        )
                    if mo % 2 == 0:
                        nc.vector.tensor_copy(out2_sb[:, cp:cp + 2, mo, :], o2[:])
                    else:
                        nc.scalar.copy(out2_sb[:, cp:cp + 2, mo, :], o2[:])
                for j in range(NCORE):
                    dst = a2a_in[cpi][j].rearrange("(cl p) b -> p cl b", cl=2)
                    if j < KN:
                        nc.gpsimd.dma_start(dst, out2_sb[:, cp:cp + 2, j, :])
                    else:
                        nc.gpsimd.dma_start(dst, zpad[:])
                nc.gpsimd.collective_compute(
                    "AllToAll", ALU.bypass,
                    replica_groups=[list(range(NCORE))],
                    ins=[a2a_in[cpi].opt()], outs=[a2a_out[cpi].opt()],
                )
                ap = a2a_out[cpi][:].rearrange("k (cl g s) b -> g (k cl) s b",
                                               cl=2, g=NG)
                for ng in range(NG):
                    nc.sync.dma_start(r_sb[cpi][ng * 16:(ng + 1) * 16, :, :],
                                      ap[ng])

            # ---- mix part 0 (overlaps second A2A transfer) ----------------
            g0 = big.tile([128, NS, B], bf16)
            for s0 in range(0, NS, 2):
                pm = psm.tile([128, 2 * B], f32, tag="psm")
                nc.tensor.matmul(pm[:], w2k0_sb[:], r_sb[0][:, s0:s0 + 2, :])
                nc.scalar.copy(g0[:, s0:s0 + 2, :], pm[:])

            # ---- mix part 1 + elu + fc1, pipelined per 2-node chunk -------
            h2_sb = big.tile([128, NS, B], bf16)
            zps = [psf.tile([128, H], f32, tag="psf", name=f"zp{i}")
                   for i in range(2)]

            def fc1_mms(s):
                for bh in range(2):
                    nc.tensor.matmul(
                        zps[bh][:],
                        h2_sb[:, s, bh * 128:(bh + 1) * 128],
                        wf1_sb[:, s, :],
                        start=(s == 0), stop=(s == NS - 1),
                    )

            for i, s0 in enumerate(range(0, NS, 2)):
                pm = psm.tile([128, 2 * B], f32, tag="psm")
                nc.tensor.matmul(pm[:], w2k1_sb[:], r_sb[1][:, s0:s0 + 2, :])
                nc.vector.tensor_tensor(pm[:], pm[:],
                                        g0[:, s0:s0 + 2, :], ALU.add)
                e = ework.tile([128, 2 * B], f32)
                nc.scalar.activation(e[:], pm[:], AF.Exp, bias=b2k_sb[:, 0:1])
                r = rwork.tile([128, 2 * B], f32)
                nc.scalar.activation(r[:], pm[:], AF.Relu, bias=b2k_sb[:, 0:1])
                nc.vector.tensor_scalar(e[:], e[:], 1.0, -1.0,
                                        ALU.min, ALU.add)
                nc.gpsimd.tensor_tensor(h2_sb[:, s0:s0 + 2, :], e[:],
                                        r[:], ALU.add)
                if i > 0:
                    fc1_mms(s0 - 2)
                    fc1_mms(s0 - 1)
            fc1_mms(NS - 2)
            fc1_mms(NS - 1)

            # ---- +bf1/8, bf16 ReduceScatter over batch blocks -------------
            zsb = small.tile([128, 2, H], bf16)
            for bh in range(2):
                nc.vector.tensor_tensor(zsb[:, bh, :], zps[bh][:],
                                        bf1b[:], ALU.add)
            rs_in = dram.tile([NCORE, BPC, H], bf16)
            rs_out = dram.tile([BPC, H], bf16)
            for j in range(NCORE):
                nc.gpsimd.dma_start(
                    rs_in[j],
                    zsb[(j % 4) * BPC:(j % 4 + 1) * BPC, j // 4, :])
            nc.gpsimd.collective_compute(
                "ReduceScatter", ALU.add,
                replica_groups=[list(range(NCORE))],
                ins=[rs_in.opt()], outs=[rs_out.opt()],
            )

            # ---- relu, FC2 (+bf2), softmax --------------------------------
            zl = small.tile([BPC, H], bf16)
            nc.sync.dma_start(zl[:], rs_out[:])
            zr = small.tile([BPC, H], f32)
            nc.scalar.activation(zr[:], zl[:], AF.Relu)

            zrT = small.tile([128, HJ, BPC], bf16)
            for hj in range(HJ):
                pt = ps2.tile([128, BPC], f32, tag="ps2")
                nc.tensor.transpose(pt[:], zr[0:BPC, hj * 128:(hj + 1) * 128],
                                    id_sb[:])
                nc.scalar.copy(zrT[:, hj, :], pt[:])

            ps_o = ps1.tile([BPC, N_OUT], f32, tag="ps1")
            for hj in range(HJ):
                nc.tensor.matmul(ps_o[:], zrT[:, hj, :], wf2_sb[:, hj, :],
                                 start=(hj == 0), stop=False)
            nc.tensor.matmul(ps_o[:], ones[0:1, 0:BPC], bf2_sb[:],
                             start=False, stop=True)

            ex = small.tile([BPC, N_OUT], f32)
            sm = small.tile([BPC, 1], f32)
            nc.scalar.activation(ex[:], ps_o[:], AF.Exp, accum_out=sm[:])
            rc = small.tile([BPC, 1], f32)
            nc.vector.reciprocal(rc[:], sm[:])
            ob = small.tile([BPC, N_OUT], f32)
            nc.vector.tensor_scalar(ob[:], ex[:], rc[0:BPC, 0:1], None,
                                    ALU.mult)
            nc.sync.dma_start(out_d[:], ob[:])

    _install_wait_splitter(nc)
    return nc


_NC_CACHE = None


def _get_program():
    global _NC_CACHE
    if _NC_CACHE is None:
        _NC_CACHE = _build_program()
    return _NC_CACHE


# ---------------------------------------------------------------------------
def _prep_inputs(x, edge_row, edge_col, edge_val, W1, b1, W2, b2,
                 Wf1, bf1, Wf2, bf2):
    import ml_dtypes
    f = np.float32
    bf = ml_dtypes.bfloat16
    A = np.zeros((N, N), f)
    np.add.at(A, (np.asarray(edge_row), np.asarray(edge_col)),
              np.asarray(edge_val, f))
    AT = np.ascontiguousarray(A.T)                                  # [m, n]
    at = np.ascontiguousarray(
        AT.reshape(KN, P, N).transpose(1, 0, 2).reshape(P, KN * N)).astype(bf)

    XT = np.ascontiguousarray(np.asarray(x, f)[:, :, 0].T)          # [N, B]
    xt = np.ascontiguousarray(
        XT.reshape(KN, P, B).transpose(1, 0, 2).reshape(P, KN * B)).astype(bf)

    W1 = np.asarray(W1, f); b1 = np.asarray(b1, f)
    W2 = np.asarray(W2, f); b2 = np.asarray(b2, f)
    Wf1 = np.asarray(Wf1, f); bf1 = np.asarray(bf1, f)
    Wf2 = np.asarray(Wf2, f); bf2 = np.asarray(bf2, f)

    # mix weights for the two channel-pair A2A halves: receive partition
    # (ng, k, cl) holds source core k's channel 4k+cl (half 0) / 4k+2+cl
    # (half 1); lhsT[(ng,k,cl), (ng',c')] = delta(ng,ng') * W2[c, c'].
    idx0 = np.array([4 * k + cl for k in range(NCORE) for cl in range(2)])
    w2k0 = np.kron(np.eye(NG, dtype=f), W2[idx0, :]).astype(bf)     # [64,128]
    w2k1 = np.kron(np.eye(NG, dtype=f), W2[idx0 + 2, :]).astype(bf)
    b2k = np.tile(b2, NG).reshape(128, 1).astype(f)

    # FC1: core k's K-chunk s holds flat rows (n=112k+ng*28+s)*C + c' at
    # partition p = ng*C + c'; rows for pad nodes (n >= 784) are zero.
    Wf1_pad = np.zeros((NPAD, C, H), f)
    Wf1_pad[:N] = Wf1.reshape(N, C, H)

    bf18 = (bf1 / NCORE).reshape(1, H).astype(f)
    wf2_l = np.ascontiguousarray(
        Wf2.reshape(HJ, 128, N_OUT).transpose(1, 0, 2).reshape(
            128, HJ * N_OUT)).astype(bf)
    bf2_l = bf2.reshape(1, N_OUT).astype(bf)
    idm = np.eye(BPC, dtype=f)

    in_maps = []
    for k in range(NCORE):
        wb = np.concatenate([W1[0, k * CPC:(k + 1) * CPC],
                             b1[k * CPC:(k + 1) * CPC]]).reshape(1, 2 * CPC)
        # [NG, NS, C, H] -> chunk s, partition (ng, c')
        wk = Wf1_pad[k * P:(k + 1) * P].reshape(NG, NS, C, H)
        wf1_l = np.ascontiguousarray(
            wk.transpose(1, 0, 2, 3).reshape(NS * 128, H)).astype(bf)
        in_maps.append({
            "at": at, "xt": xt, "wf1": wf1_l,
            "wb": np.ascontiguousarray(wb.astype(f)),
            "w2k0": w2k0, "w2k1": w2k1, "b2k": b2k,
            "bf18": bf18, "wf2": wf2_l, "bf2": bf2_l, "idm": idm,
        })
    return in_maps


def kernel(x, edge_row, edge_col, edge_val, W1, b1, W2, b2,
           Wf1, bf1, Wf2, bf2, **kw):
    nc = _get_program()
    in_maps = _prep_inputs(x, edge_row, edge_col, edge_val, W1, b1, W2, b2,
                           Wf1, bf1, Wf2, bf2)
    res = run_bass_kernel_spmd(nc, in_maps, list(range(NCORE)), **kw)
    out = np.concatenate([res.results[k]["out"] for k in range(NCORE)], axis=0)
    if kw.get("trace"):
        kernel.last_exec_time_ns = res.exec_time_ns
    return out.astype(np.float32)


# revision 8
# speedup vs baseline: 1.0142x; 1.0070x over previous
"""GNN message-passing net on 8 Trainium2 cores.

Reference: x:[256,784,1] -> h1 = elu(spmm(x)@W1+b1) -> h2 = elu(spmm(h1)@W2+b2)
-> flat[B, N*C] -> relu(flat@Wf1+bf1) -> softmax(z@Wf2+bf2).

Strategy (all matmul operands bf16, fp32 PSUM accumulation):
  * Densify the sparse filter A (784x784, ~1% nz) on the host; spmm becomes
    dense matmuls on the PE array.
  * F=1 makes conv1 an outer product: out1 = A @ X^T [784,256] shared by all
    channels; h1_c = elu(W1[c]*out1+b1[c]) with per-channel big-tile elu
    (elu(t)=min(exp(t),1)+relu(t)-1) over o1 staged in SBUF.
  * Conv2 spmm channel-sharded: core k computes out2_c = A @ h1_c for
    channels 4k..4k+3, full batch.  The channel->node AllToAll is split into
    FOUR node-group quarters: quarter q carries all 32 channels for nodes
    [112j+28q, 112j+28q+28) of each destination j, so the mix + elu + fc1
    work for quarter q overlaps the transfer of quarter q+1.  A tiny dummy
    collective issued at t~0 absorbs the one-time cc-stream barrier.
  * Received quarter layout packs (s%4, channel) into 128 partitions so the
    kron(I4, W2) mix matmul and the elementwise elu run at full width.
  * FC1 K-sharded with z in [batch, h] layout (F=512 matmuls); each core
    adds bf1/8 before a 2-way (h-halved) bf16 ReduceScatter over batch
    blocks, packed/unpacked with single strided DMAs.
  * Tail: relu, PE-transpose of z, FC2 (+bf2 via ones-row matmul), softmax
    without max-subtraction (logits are O(4)), Exp with fused row-sum.
"""
import json

import numpy as np

import concourse.bass as bass
import concourse.mybir as mybir
import concourse.tile as tile
from concourse.bass_utils import run_bass_kernel_spmd

B, N, F, E = 256, 784, 1, 6272
C, H, N_OUT = 32, 512, 10
NCORE = 8
CPC = C // NCORE      # 4 channels per core in conv2
P = 112               # 784 = 7 * 112
KN = N // P           # 7 node chunks
NPAD = P * NCORE      # 896 padded nodes for the node reshard
NG = 4                # node-group quarters per core block
NS = P // NG          # 28 nodes per quarter
SH = NS // 4          # 7 sh chunks per quarter (s = sh*4 + s4)
BPC = B // NCORE      # 32 batch rows per core
HJ = H // 128         # 4 h chunks

f32 = mybir.dt.float32
bf16 = mybir.dt.bfloat16
AF = mybir.ActivationFunctionType
ALU = mybir.AluOpType
AX = mybir.AxisListType


# ---------------------------------------------------------------------------
# BIR post-pass: this walrus build rejects instructions with >1 sync-wait;
# split extras onto standalone EventSemaphore instructions (same engine,
# inserted just before, so the engine stream stalls identically).
def _split_waits(bir: dict, max_waits: int = 1) -> dict:
    n = [0]
    for fn in bir.get("functions", []):
        for blk in fn.get("blocks", []):
            out = []
            for ins in blk.get("instructions", []):
                si = ins.get("sync_info") or {}
                waits = si.get("on_wait") or []
                if len(waits) > max_waits:
                    for w in waits[max_waits:]:
                        n[0] += 1
                        out.append({
                            "name": f"I-waitsplit-{n[0]}",
                            "opcode": "EventSemaphore",
                            "engine": ins["engine"],
                            "ins": [], "outs": [],
                            **({"debug": ins["debug"]} if "debug" in ins else {}),
                            "sync_info": {"on_update": [], "on_wait": [w]},
                        })
                    si = dict(si)
                    si["on_wait"] = waits[:max_waits]
                    ins = dict(ins)
                    ins["sync_info"] = si
                out.append(ins)
            blk["instructions"] = out
    return bir


def _install_wait_splitter(nc):
    orig = nc.to_json_bytes
    nc.to_json_bytes = lambda: json.dumps(_split_waits(json.loads(orig()))).encode()


# ---------------------------------------------------------------------------
def _build_program():
    nc = bass.Bass(num_devices=NCORE)

    at_d = nc.dram_tensor("at", [P, KN * N], bf16, kind="ExternalInput")
    xt_d = nc.dram_tensor("xt", [P, KN * B], bf16, kind="ExternalInput")
    wf1_d = nc.dram_tensor("wf1", [NS * 128, H], bf16, kind="ExternalInput")
    wb_d = nc.dram_tensor("wb", [1, 2 * CPC], f32, kind="ExternalInput")
    w2k_d = nc.dram_tensor("w2k", [128, 128], bf16, kind="ExternalInput")
    b2k_d = nc.dram_tensor("b2k", [128, 1], f32, kind="ExternalInput")
    bf18_d = nc.dram_tensor("bf18", [1, H], f32, kind="ExternalInput")
    wf2_d = nc.dram_tensor("wf2", [128, HJ * N_OUT], bf16, kind="ExternalInput")
    bf2_d = nc.dram_tensor("bf2", [1, N_OUT], bf16, kind="ExternalInput")
    id_d = nc.dram_tensor("idm", [BPC, BPC], f32, kind="ExternalInput")
    out_d = nc.dram_tensor("out", [BPC, N_OUT], f32, kind="ExternalOutput")

    with tile.TileContext(nc) as tc:
        with (
            tc.tile_pool(name="big", bufs=1) as big,
            tc.tile_pool(name="small", bufs=1) as small,
            tc.tile_pool(name="ework", bufs=3) as ework,
            tc.tile_pool(name="rwork", bufs=3) as rwork,
            tc.tile_pool(name="ps1", bufs=2, space="PSUM") as ps1,
            tc.tile_pool(name="ps2", bufs=2, space="PSUM") as ps2,
            tc.tile_pool(name="psm", bufs=2, space="PSUM") as psm,
            tc.tile_pool(name="psf", bufs=2, space="PSUM") as psf,
            tc.tile_pool(name="dram", bufs=1, space="DRAM") as dram,
        ):
            # ---- dummy collective: absorb cc-stream init barrier ----------
            dum_i = dram.tile([NCORE, 1, 64], bf16)
            dum_o = dram.tile([NCORE, 1, 64], bf16)
            dz = small.tile([NCORE, 64], bf16)
            nc.gpsimd.memset(dz[:], 0.0)
            nc.gpsimd.dma_start(dum_i[:].rearrange("k o b -> (k o) b"), dz[:])
            nc.gpsimd.collective_compute(
                "AllToAll", ALU.bypass,
                replica_groups=[list(range(NCORE))],
                ins=[dum_i.opt()], outs=[dum_o.opt()],
            )

            # ---- resident inputs ------------------------------------------
            at_sb = big.tile([P, KN, N], bf16)
            xt_sb = big.tile([P, KN, B], bf16)
            wf1_sb = big.tile([128, NS, H], bf16)
            wb_sb = small.tile([1, 2 * CPC], f32)
            w2k_sb = small.tile([128, 128], bf16)
            b2k_sb = small.tile([128, 1], f32)
            bf18_sb = small.tile([1, H], f32)
            wf2_sb = small.tile([128, HJ, N_OUT], bf16)
            bf2_sb = small.tile([1, N_OUT], bf16)
            id_sb = small.tile([BPC, BPC], f32)
            ones = small.tile([1, 128], bf16)
            ones_f = small.tile([1, 128], f32)

            xt_ap = xt_d[:].rearrange("p (k b) -> p k b", k=KN)
            at_ap = at_d[:].rearrange("p (k n) -> p k n", k=KN)
            # sync queue order = availability order for conv1 startup
            nc.sync.dma_start(xt_sb[:, 0:4, :], xt_ap[:, 0:4, :])
            nc.sync.dma_start(at_sb[:, 0:4, :], at_ap[:, 0:4, :])
            nc.sync.dma_start(wb_sb[:], wb_d[:])
            nc.sync.dma_start(xt_sb[:, 4:KN, :], xt_ap[:, 4:KN, :])
            nc.sync.dma_start(at_sb[:, 4:KN, :], at_ap[:, 4:KN, :])
            nc.sync.dma_start(w2k_sb[:], w2k_d[:])
            nc.sync.dma_start(b2k_sb[:], b2k_d[:])
            nc.sync.dma_start(bf18_sb[:], bf18_d[:])
            nc.sync.dma_start(wf2_sb[:], wf2_d[:].rearrange("p (j o) -> p j o", j=HJ))
            nc.sync.dma_start(bf2_sb[:], bf2_d[:])
            nc.sync.dma_start(id_sb[:], id_d[:])
            nc.sync.dma_start(wf1_sb[:],
                              wf1_d[:].rearrange("(s p) h -> p s h", p=128))
            nc.vector.memset(ones[:], 1.0)
            nc.vector.memset(ones_f[:], 1.0)

            # ---- broadcast W1/b1 channel scalars across partitions --------
            ps_bc = ps1.tile([128, 2 * CPC], f32, tag="ps1")
            nc.tensor.matmul(ps_bc[:], ones_f[0:1, 0:128], wb_sb[:])
            wband = small.tile([128, 2 * CPC], f32)
            nc.vector.tensor_copy(wband[:], ps_bc[:])

            # ---- conv1: o1 = A @ X^T staged to SBUF -----------------------
            o1s = big.tile([P, KN, B], f32)
            for mc in range(KN):
                o1 = ps1.tile([P, B], f32, tag="ps1")
                for kc in range(KN):
                    nc.tensor.matmul(
                        o1[:],
                        at_sb[:, kc, mc * P:(mc + 1) * P],
                        xt_sb[:, kc, :],
                        start=(kc == 0), stop=(kc == KN - 1),
                    )
                if mc % 2 == 0:
                    nc.scalar.copy(o1s[:, mc, :], o1[:])
                else:
                    nc.vector.tensor_copy(o1s[:, mc, :], o1[:])

            # bf1/8 broadcast across partitions (needed only at fc1 time)
            ps_bf = psm.tile([128, H], f32, tag="psm")
            nc.tensor.matmul(ps_bf[:], ones_f[0:1, 0:128], bf18_sb[:])
            bf1b = small.tile([128, H], f32)
            nc.vector.tensor_copy(bf1b[:], ps_bf[:])

            # ---- conv1 elu: h1_c = elu(w1_c * o1 + b1_c), big tiles -------
            h1_sb = big.tile([P, CPC, KN, B], bf16)
            for c, k0, k1 in ((0, 0, 4), (1, 0, 4), (0, 4, KN), (1, 4, KN),
                              (2, 0, 4), (3, 0, 4), (2, 4, KN), (3, 4, KN)):
                fw = (k1 - k0) * B
                sc = wband[0:P, c:c + 1]
                bi = wband[0:P, CPC + c:CPC + c + 1]
                e = ework.tile([P, 4 * B], f32)
                nc.scalar.activation(e[:, 0:fw], o1s[:, k0:k1, :], AF.Exp,
                                     bias=bi, scale=sc)
                r = rwork.tile([P, 4 * B], f32)
                nc.scalar.activation(r[:, 0:fw], o1s[:, k0:k1, :], AF.Relu,
                                     bias=bi, scale=sc)
                nc.vector.tensor_scalar(e[:, 0:fw], e[:, 0:fw],
                                        1.0, -1.0, ALU.min, ALU.add)
                nc.vector.tensor_tensor(h1_sb[:, c, k0:k1, :], e[:, 0:fw],
                                        r[:, 0:fw], ALU.add)

            # ---- conv2 spmm: out2_c = A @ h1_c (mo-major) -----------------
            out2_sb = big.tile([P, CPC, KN, B], bf16)
            for mo in range(KN):
                for cpi, cp in enumerate((0, 2)):
                    o2 = ps2.tile([P, 2, B], f32, tag="ps2")
                    for kc in range(KN):
                        nc.tensor.matmul(
                            o2[:],
                            at_sb[:, kc, mo * P:(mo + 1) * P],
                            h1_sb[:, cp:cp + 2, kc, :],
                            start=(kc == 0), stop=(kc == KN - 1),
                        )
                    if (2 * mo + cpi) % 2 == 0:
                        nc.vector.tensor_copy(out2_sb[:, cp:cp + 2, mo, :], o2[:])
                    else:
                        nc.scalar.copy(out2_sb[:, cp:cp + 2, mo, :], o2[:])

            # ---- AllToAll x4: node-group quarters -------------------------
            # quarter q block j carries [(cl, s), b] = my 4 channels for
            # nodes 112j + 28q + s; receiver repacks to [(s4, c), sh, b].
            zpad = small.tile([NS, CPC, B], bf16)
            nc.vector.memset(zpad[:], 0.0)
            a2a_in = [dram.tile([NCORE, CPC * NS, B], bf16, name=f"a2ai{q}")
                      for q in range(NG)]
            a2a_out = [dram.tile([NCORE, CPC * NS, B], bf16, name=f"a2ao{q}")
                       for q in range(NG)]
            r_sb = [big.tile([128, SH, B], bf16, name=f"rsb{q}")
                    for q in range(NG)]
            for q in range(NG):
                # zero pad block (node block 7 does not exist): written early
                nc.gpsimd.dma_start(
                    a2a_in[q][KN].rearrange("(cl p) b -> p cl b", cl=CPC),
                    zpad[:])
            for q in range(NG):
                for j in range(KN):
                    nc.gpsimd.dma_start(
                        a2a_in[q][j].rearrange("(cl p) b -> p cl b", cl=CPC),
                        out2_sb[q * NS:(q + 1) * NS, :, j, :])
                nc.gpsimd.collective_compute(
                    "AllToAll", ALU.bypass,
                    replica_groups=[list(range(NCORE))],
                    ins=[a2a_in[q].opt()], outs=[a2a_out[q].opt()],
                )
                src = a2a_out[q][:].rearrange(
                    "k (cl sh s4) b -> s4 (k cl) sh b", cl=CPC, s4=4)
                for s4 in range(4):
                    nc.sync.dma_start(r_sb[q][s4 * 32:(s4 + 1) * 32, :, :],
                                      src[s4])

            # ---- mix + elu + fc1, pipelined per quarter chunk -------------
            h2_sb = big.tile([128, NS, B], bf16)
            zps = [psf.tile([128, H], f32, tag="psf", name=f"zp{i}")
                   for i in range(2)]

            def fc1_mms(t):
                for bh in range(2):
                    nc.tensor.matmul(
                        zps[bh][:],
                        h2_sb[:, t, bh * 128:(bh + 1) * 128],
                        wf1_sb[:, t, :],
                        start=(t == 0), stop=(t == NS - 1),
                    )

            chunks = [(q, sh0, min(2, SH - sh0))
                      for q in range(NG) for sh0 in range(0, SH, 2)]
            done = []
            for q, sh0, shw in chunks:
                fw = shw * B
                t0c = q * SH + sh0
                pm = psm.tile([128, 2 * B], f32, tag="psm")
                nc.tensor.matmul(pm[:, 0:fw], w2k_sb[:],
                                 r_sb[q][:, sh0:sh0 + shw, :])
                e = ework.tile([128, 2 * B], f32)
                nc.scalar.activation(e[:, 0:fw], pm[:, 0:fw], AF.Exp,
                                     bias=b2k_sb[:, 0:1])
                r = rwork.tile([128, 2 * B], f32)
                nc.scalar.activation(r[:, 0:fw], pm[:, 0:fw], AF.Relu,
                                     bias=b2k_sb[:, 0:1])
                nc.vector.tensor_scalar(e[:, 0:fw], e[:, 0:fw],
                                        1.0, -1.0, ALU.min, ALU.add)
                nc.vector.tensor_tensor(h2_sb[:, t0c:t0c + shw, :],
                                        e[:, 0:fw], r[:, 0:fw], ALU.add)
                while done:
                    fc1_mms(done.pop(0))
                done.extend(range(t0c, t0c + shw))
            while done:
                fc1_mms(done.pop(0))

            # ---- +bf1/8, 2-way bf16 ReduceScatter over batch blocks -------
            zsb = small.tile([128, 2, H], bf16)
            for bh in range(2):
                nc.vector.tensor_tensor(zsb[:, bh, :], zps[bh][:],
                                        bf1b[:], ALU.add)
            rs_in = [dram.tile([NCORE, BPC, H // 2], bf16, name=f"rsi{i}")
                     for i in range(2)]
            rs_out = [dram.tile([BPC, H // 2], bf16, name=f"rso{i}")
                      for i in range(2)]
            zl = small.tile([BPC, H], bf16)
            for hh in range(2):
                dst = rs_in[hh][:].rearrange("(jh jl) b h -> (jl b) jh h",
                                             jh=2)
                nc.sync.dma_start(
                    dst, zsb[:, :, hh * (H // 2):(hh + 1) * (H // 2)])
                nc.gpsimd.collective_compute(
                    "ReduceScatter", ALU.add,
                    replica_groups=[list(range(NCORE))],
                    ins=[rs_in[hh].opt()], outs=[rs_out[hh].opt()],
                )
                nc.sync.dma_start(
                    zl[:, hh * (H // 2):(hh + 1) * (H // 2)], rs_out[hh][:])

            # ---- relu, FC2 (+bf2), softmax --------------------------------
            zr = small.tile([BPC, H], f32)
            for hh in range(2):
                nc.scalar.activation(
                    zr[:, hh * (H // 2):(hh + 1) * (H // 2)],
                    zl[:, hh * (H // 2):(hh + 1) * (H // 2)], AF.Relu)

            zrT = small.tile([128, HJ, BPC], bf16)
            for hj in range(HJ):
                pt = ps2.tile([128, BPC], f32, tag="ps2")
                nc.tensor.transpose(pt[:], zr[0:BPC, hj * 128:(hj + 1) * 128],
                                    id_sb[:])
                nc.scalar.copy(zrT[:, hj, :], pt[:])

            ps_o = ps1.tile([BPC, N_OUT], f32, tag="ps1")
            for hj in range(HJ):
                nc.tensor.matmul(ps_o[:], zrT[:, hj, :], wf2_sb[:, hj, :],
                                 start=(hj == 0), stop=False)
            nc.tensor.matmul(ps_o[:], ones[0:1, 0:BPC], bf2_sb[:],
                             start=False, stop=True)

            ex = small.tile([BPC, N_OUT], f32)
            sm = small.tile([BPC, 1], f32)
            nc.scalar.activation(ex[:], ps_o[:], AF.Exp, accum_out=sm[:])
            rc = small.tile([BPC, 1], f32)
            nc.vector.reciprocal(rc[:], sm[:])
            ob = small.tile([BPC, N_OUT], f32)
            nc.vector.tensor_scalar(ob[:], ex[:], rc[0:BPC, 0:1], None,
                                    ALU.mult)
            nc.sync.dma_start(out_d[:], ob[:])

    _install_wait_splitter(nc)
    return nc


_NC_CACHE = None


def _get_program():
    global _NC_CACHE
    if _NC_CACHE is None:
        _NC_CACHE = _build_program()
    return _NC_CACHE


# ---------------------------------------------------------------------------
def _prep_inputs(x, edge_row, edge_col, edge_val, W1, b1, W2, b2,
                 Wf1, bf1, Wf2, bf2):
    import ml_dtypes
    f = np.float32
    bf = ml_dtypes.bfloat16
    A = np.zeros((N, N), f)
    np.add.at(A, (np.asarray(edge_row), np.asarray(edge_col)),
              np.asarray(edge_val, f))
    AT = np.ascontiguousarray(A.T)                                  # [m, n]
    at = np.ascontiguousarray(
        AT.reshape(KN, P, N).transpose(1, 0, 2).reshape(P, KN * N)).astype(bf)

    XT = np.ascontiguousarray(np.asarray(x, f)[:, :, 0].T)          # [N, B]
    xt = np.ascontiguousarray(
        XT.reshape(KN, P, B).transpose(1, 0, 2).reshape(P, KN * B)).astype(bf)

    W1 = np.asarray(W1, f); b1 = np.asarray(b1, f)
    W2 = np.asarray(W2, f); b2 = np.asarray(b2, f)
    Wf1 = np.asarray(Wf1, f); bf1 = np.asarray(bf1, f)
    Wf2 = np.asarray(Wf2, f); bf2 = np.asarray(bf2, f)

    # mix weight: lhsT[(s4,c),(s4',c')] = delta(s4,s4') * W2[c,c']
    w2k = np.kron(np.eye(4, dtype=f), W2).astype(bf)                # [128,128]
    b2k = np.tile(b2, 4).reshape(128, 1).astype(f)

    # FC1: core k's K-chunk t = q*SH+sh holds flat rows for node
    # n = 112k + 28q + 4sh + s4, channel c', at partition p = s4*C + c';
    # rows for pad nodes (n >= 784) are zero.
    Wf1_pad = np.zeros((NPAD, C, H), f)
    Wf1_pad[:N] = Wf1.reshape(N, C, H)

    bf18 = (bf1 / NCORE).reshape(1, H).astype(f)
    wf2_l = np.ascontiguousarray(
        Wf2.reshape(HJ, 128, N_OUT).transpose(1, 0, 2).reshape(
            128, HJ * N_OUT)).astype(bf)
    bf2_l = bf2.reshape(1, N_OUT).astype(bf)
    idm = np.eye(BPC, dtype=f)

    in_maps = []
    for k in range(NCORE):
        wb = np.concatenate([W1[0, k * CPC:(k + 1) * CPC],
                             b1[k * CPC:(k + 1) * CPC]]).reshape(1, 2 * CPC)
        # [q, sh, s4, c, H]: chunk (q, sh), partition (s4, c)
        wk = Wf1_pad[k * P:(k + 1) * P].reshape(NG, SH, 4, C, H)
        wf1_l = np.ascontiguousarray(wk.reshape(NS * 128, H)).astype(bf)
        in_maps.append({
            "at": at, "xt": xt, "wf1": wf1_l,
            "wb": np.ascontiguousarray(wb.astype(f)),
            "w2k": w2k, "b2k": b2k,
            "bf18": bf18, "wf2": wf2_l, "bf2": bf2_l, "idm": idm,
        })
    return in_maps


def kernel(x, edge_row, edge_col, edge_val, W1, b1, W2, b2,
           Wf1, bf1, Wf2, bf2, **kw):
    nc = _get_program()
    in_maps = _prep_inputs(x, edge_row, edge_col, edge_val, W1, b1, W2, b2,
                           Wf1, bf1, Wf2, bf2)
    res = run_bass_kernel_spmd(nc, in_maps, list(range(NCORE)), **kw)
    out = np.concatenate([res.results[k]["out"] for k in range(NCORE)], axis=0)
    if kw.get("trace"):
        kernel.last_exec_time_ns = res.exec_time_ns
    return out.astype(np.float32)


# revision 10
# speedup vs baseline: 1.1786x; 1.1621x over previous
"""GNN message-passing net on 8 Trainium2 cores.

Reference: x:[256,784,1] -> h1 = elu(spmm(x)@W1+b1) -> h2 = elu(spmm(h1)@W2+b2)
-> flat[B, N*C] -> relu(flat@Wf1+bf1) -> softmax(z@Wf2+bf2).

Strategy (all matmul operands bf16, fp32 PSUM accumulation):
  * Densify the sparse filter A (784x784, ~1% nz) on the host; spmm becomes
    dense matmuls on the PE array.
  * F=1 makes conv1 an outer product: out1 = A @ X^T [784,256] shared by all
    channels; h1_c = elu(W1[c]*out1+b1[c]) with per-channel big-tile elu
    (elu(t)=min(exp(t),1)+relu(t)-1) over o1 staged in SBUF.
  * Conv2 spmm channel-sharded: core k computes out2_c = A @ h1_c for
    channels 4k..4k+3, full batch.  The channel->node AllToAll is split into
    FOUR node-group quarters: quarter q carries all 32 channels for nodes
    [112j+28q, 112j+28q+28) of each destination j, so the mix + elu + fc1
    work for quarter q overlaps the transfer of quarter q+1.  A tiny dummy
    collective issued at t~0 absorbs the one-time cc-stream barrier.
  * Received quarter layout packs (s%4, channel) into 128 partitions so the
    kron(I4, W2) mix matmul and the elementwise elu run at full width.
  * FC1 K-sharded with z in [batch, h] layout (F=512 matmuls); each core
    adds bf1/8 before a 2-way (h-halved) bf16 ReduceScatter over batch
    blocks, packed/unpacked with single strided DMAs.
  * Tail: relu, PE-transpose of z, FC2 (+bf2 via ones-row matmul), softmax
    without max-subtraction (logits are O(4)), Exp with fused row-sum.
"""
import json

import numpy as np

import concourse.bass as bass
import concourse.mybir as mybir
import concourse.tile as tile
from concourse.bass_utils import run_bass_kernel_spmd

B, N, F, E = 256, 784, 1, 6272
C, H, N_OUT = 32, 512, 10
NCORE = 8
CPC = C // NCORE      # 4 channels per core in conv2
P = 112               # 784 = 7 * 112
KN = N // P           # 7 node chunks
NPAD = P * NCORE      # 896 padded nodes for the node reshard
NG = 4                # node-group quarters per core block
NS = P // NG          # 28 nodes per quarter
SH = NS // 4          # 7 sh chunks per quarter (s = sh*4 + s4)
BPC = B // NCORE      # 32 batch rows per core
HJ = H // 128         # 4 h chunks

f32 = mybir.dt.float32
bf16 = mybir.dt.bfloat16
AF = mybir.ActivationFunctionType
ALU = mybir.AluOpType
AX = mybir.AxisListType


# ---------------------------------------------------------------------------
# BIR post-pass: this walrus build rejects instructions with >1 sync-wait;
# split extras onto standalone EventSemaphore instructions (same engine,
# inserted just before, so the engine stream stalls identically).
def _split_waits(bir: dict, max_waits: int = 1) -> dict:
    n = [0]
    for fn in bir.get("functions", []):
        for blk in fn.get("blocks", []):
            out = []
            for ins in blk.get("instructions", []):
                si = ins.get("sync_info") or {}
                waits = si.get("on_wait") or []
                if len(waits) > max_waits:
                    for w in waits[max_waits:]:
                        n[0] += 1
                        out.append({
                            "name": f"I-waitsplit-{n[0]}",
                            "opcode": "EventSemaphore",
                            "engine": ins["engine"],
                            "ins": [], "outs": [],
                            **({"debug": ins["debug"]} if "debug" in ins else {}),
                            "sync_info": {"on_update": [], "on_wait": [w]},
                        })
                    si = dict(si)
                    si["on_wait"] = waits[:max_waits]
                    ins = dict(ins)
                    ins["sync_info"] = si
                out.append(ins)
            blk["instructions"] = out
    return bir


def _install_wait_splitter(nc):
    orig = nc.to_json_bytes
    nc.to_json_bytes = lambda: json.dumps(_split_waits(json.loads(orig()))).encode()


# ---------------------------------------------------------------------------
def _build_program():
    nc = bass.Bass(num_devices=NCORE)

    at_d = nc.dram_tensor("at", [P, KN * N], bf16, kind="ExternalInput")
    xt_d = nc.dram_tensor("xt", [P, KN * B], bf16, kind="ExternalInput")
    wf1_d = nc.dram_tensor("wf1", [NS * 128, H], bf16, kind="ExternalInput")
    wb_d = nc.dram_tensor("wb", [1, 2 * CPC], f32, kind="ExternalInput")
    w2k_d = nc.dram_tensor("w2k", [128, 128], bf16, kind="ExternalInput")
    b2k_d = nc.dram_tensor("b2k", [128, 1], f32, kind="ExternalInput")
    bf18_d = nc.dram_tensor("bf18", [1, H], f32, kind="ExternalInput")
    wf2_d = nc.dram_tensor("wf2", [128, HJ * N_OUT], bf16, kind="ExternalInput")
    bf2_d = nc.dram_tensor("bf2", [1, N_OUT], bf16, kind="ExternalInput")
    id_d = nc.dram_tensor("idm", [BPC, BPC], f32, kind="ExternalInput")
    out_d = nc.dram_tensor("out", [BPC, N_OUT], f32, kind="ExternalOutput")

    with tile.TileContext(nc) as tc:
        with (
            tc.tile_pool(name="big", bufs=1) as big,
            tc.tile_pool(name="small", bufs=1) as small,
            tc.tile_pool(name="ework", bufs=3) as ework,
            tc.tile_pool(name="rwork", bufs=3) as rwork,
            tc.tile_pool(name="ps1", bufs=2, space="PSUM") as ps1,
            tc.tile_pool(name="ps2", bufs=2, space="PSUM") as ps2,
            tc.tile_pool(name="psm", bufs=2, space="PSUM") as psm,
            tc.tile_pool(name="psf", bufs=2, space="PSUM") as psf,
            tc.tile_pool(name="dram", bufs=1, space="DRAM") as dram,
        ):
            # ---- resident inputs ------------------------------------------
            at_sb = big.tile([P, KN, N], bf16)
            xt_sb = big.tile([P, KN, B], bf16)
            wf1_sb = big.tile([128, NS, H], bf16)
            wb_sb = small.tile([1, 2 * CPC], f32)
            w2k_sb = small.tile([128, 128], bf16)
            b2k_sb = small.tile([128, 1], f32)
            bf18_sb = small.tile([1, H], f32)
            wf2_sb = small.tile([128, HJ, N_OUT], bf16)
            bf2_sb = small.tile([1, N_OUT], bf16)
            id_sb = small.tile([BPC, BPC], f32)
            ones = small.tile([1, 128], bf16)
            ones_f = small.tile([1, 128], f32)

            xt_ap = xt_d[:].rearrange("p (k b) -> p k b", k=KN)
            at_ap = at_d[:].rearrange("p (k n) -> p k n", k=KN)
            # sync queue order = availability order for conv1 startup
            nc.sync.dma_start(xt_sb[:, 0:4, :], xt_ap[:, 0:4, :])
            nc.sync.dma_start(at_sb[:, 0:4, :], at_ap[:, 0:4, :])
            nc.sync.dma_start(wb_sb[:], wb_d[:])
            nc.sync.dma_start(xt_sb[:, 4:KN, :], xt_ap[:, 4:KN, :])
            nc.sync.dma_start(at_sb[:, 4:KN, :], at_ap[:, 4:KN, :])
            nc.sync.dma_start(w2k_sb[:], w2k_d[:])
            nc.sync.dma_start(b2k_sb[:], b2k_d[:])
            nc.sync.dma_start(bf18_sb[:], bf18_d[:])
            nc.sync.dma_start(wf2_sb[:], wf2_d[:].rearrange("p (j o) -> p j o", j=HJ))
            nc.sync.dma_start(bf2_sb[:], bf2_d[:])
            nc.sync.dma_start(id_sb[:], id_d[:])
            nc.sync.dma_start(wf1_sb[:],
                              wf1_d[:].rearrange("(s p) h -> p s h", p=128))
            nc.vector.memset(ones[:], 1.0)
            nc.vector.memset(ones_f[:], 1.0)

            # ---- broadcast W1/b1 channel scalars across partitions --------
            ps_bc = ps1.tile([128, 2 * CPC], f32, tag="ps1")
            nc.tensor.matmul(ps_bc[:], ones_f[0:1, 0:128], wb_sb[:])
            wband = small.tile([128, 2 * CPC], f32)
            nc.vector.tensor_copy(wband[:], ps_bc[:])

            # ---- conv1: o1 = A @ X^T staged to SBUF -----------------------
            o1s = big.tile([P, KN, B], f32)
            for mc in range(KN):
                o1 = ps1.tile([P, B], f32, tag="ps1")
                for kc in range(KN):
                    nc.tensor.matmul(
                        o1[:],
                        at_sb[:, kc, mc * P:(mc + 1) * P],
                        xt_sb[:, kc, :],
                        start=(kc == 0), stop=(kc == KN - 1),
                    )
                if mc % 2 == 0:
                    nc.scalar.copy(o1s[:, mc, :], o1[:])
                else:
                    nc.vector.tensor_copy(o1s[:, mc, :], o1[:])

            # bf1/8 broadcast across partitions (needed only at fc1 time)
            ps_bf = psm.tile([128, H], f32, tag="psm")
            nc.tensor.matmul(ps_bf[:], ones_f[0:1, 0:128], bf18_sb[:])
            bf1b = small.tile([128, H], f32)
            nc.vector.tensor_copy(bf1b[:], ps_bf[:])

            # ---- conv1 elu: h1_c = elu(w1_c * o1 + b1_c), big tiles -------
            h1_sb = big.tile([P, CPC, KN, B], bf16)
            for c, k0, k1 in ((0, 0, 4), (1, 0, 4), (0, 4, KN), (1, 4, KN),
                              (2, 0, 4), (3, 0, 4), (2, 4, KN), (3, 4, KN)):
                fw = (k1 - k0) * B
                sc = wband[0:P, c:c + 1]
                bi = wband[0:P, CPC + c:CPC + c + 1]
                e = ework.tile([P, 4 * B], f32)
                nc.scalar.activation(e[:, 0:fw], o1s[:, k0:k1, :], AF.Exp,
                                     bias=bi, scale=sc)
                r = rwork.tile([P, 4 * B], f32)
                nc.scalar.activation(r[:, 0:fw], o1s[:, k0:k1, :], AF.Relu,
                                     bias=bi, scale=sc)
                nc.vector.tensor_scalar(e[:, 0:fw], e[:, 0:fw],
                                        1.0, -1.0, ALU.min, ALU.add)
                nc.vector.tensor_tensor(h1_sb[:, c, k0:k1, :], e[:, 0:fw],
                                        r[:, 0:fw], ALU.add)

            # ---- conv2 spmm: out2_c = A @ h1_c (mo-major) -----------------
            out2_sb = big.tile([P, CPC, KN, B], bf16)
            for mo in range(KN):
                for cpi, cp in enumerate((0, 2)):
                    o2 = ps2.tile([P, 2, B], f32, tag="ps2")
                    for kc in range(KN):
                        nc.tensor.matmul(
                            o2[:],
                            at_sb[:, kc, mo * P:(mo + 1) * P],
                            h1_sb[:, cp:cp + 2, kc, :],
                            start=(kc == 0), stop=(kc == KN - 1),
                        )
                    if (2 * mo + cpi) % 2 == 0:
                        nc.vector.tensor_copy(out2_sb[:, cp:cp + 2, mo, :], o2[:])
                    else:
                        nc.scalar.copy(out2_sb[:, cp:cp + 2, mo, :], o2[:])

            # ---- AllToAll x4: node-group quarters -------------------------
            # quarter q block j carries [(cl, s), b] = my 4 channels for
            # nodes 112j + 28q + s; receiver repacks to [(s4, c), sh, b].
            zpad = small.tile([NS, CPC, B], bf16)
            nc.vector.memset(zpad[:], 0.0)
            a2a_in = [dram.tile([NCORE, CPC * NS, B], bf16, name=f"a2ai{q}")
                      for q in range(NG)]
            a2a_out = [dram.tile([NCORE, CPC * NS, B], bf16, name=f"a2ao{q}")
                       for q in range(NG)]
            r_sb = [big.tile([128, SH, B], bf16, name=f"rsb{q}")
                    for q in range(NG)]
            for q in range(NG):
                # zero pad block (node block 7 does not exist): written early
                nc.gpsimd.dma_start(
                    a2a_in[q][KN].rearrange("(cl p) b -> p cl b", cl=CPC),
                    zpad[:])
            for q in range(NG):
                for j in range(KN):
                    nc.gpsimd.dma_start(
                        a2a_in[q][j].rearrange("(cl p) b -> p cl b", cl=CPC),
                        out2_sb[q * NS:(q + 1) * NS, :, j, :])
                nc.gpsimd.collective_compute(
                    "AllToAll", ALU.bypass,
                    replica_groups=[list(range(NCORE))],
                    ins=[a2a_in[q].opt()], outs=[a2a_out[q].opt()],
                )
                src = a2a_out[q][:].rearrange(
                    "k (cl sh s4) b -> s4 (k cl) sh b", cl=CPC, s4=4)
                for s4 in range(4):
                    nc.sync.dma_start(r_sb[q][s4 * 32:(s4 + 1) * 32, :, :],
                                      src[s4])

            # ---- mix + elu + fc1, pipelined per quarter chunk -------------
            h2_sb = big.tile([128, NS, B], bf16)
            zps = [psf.tile([128, H], f32, tag="psf", name=f"zp{i}")
                   for i in range(2)]

            def fc1_mms(t):
                for bh in range(2):
                    nc.tensor.matmul(
                        zps[bh][:],
                        h2_sb[:, t, bh * 128:(bh + 1) * 128],
                        wf1_sb[:, t, :],
                        start=(t == 0), stop=(t == NS - 1),
                    )

            chunks = [(q, sh0, min(2, SH - sh0))
                      for q in range(NG) for sh0 in range(0, SH, 2)]
            done = []
            for q, sh0, shw in chunks:
                fw = shw * B
                t0c = q * SH + sh0
                pm = psm.tile([128, 2 * B], f32, tag="psm")
                nc.tensor.matmul(pm[:, 0:fw], w2k_sb[:],
                                 r_sb[q][:, sh0:sh0 + shw, :])
                e = ework.tile([128, 2 * B], f32)
                nc.scalar.activation(e[:, 0:fw], pm[:, 0:fw], AF.Exp,
                                     bias=b2k_sb[:, 0:1])
                r = rwork.tile([128, 2 * B], f32)
                nc.scalar.activation(r[:, 0:fw], pm[:, 0:fw], AF.Relu,
                                     bias=b2k_sb[:, 0:1])
                nc.vector.tensor_scalar(e[:, 0:fw], e[:, 0:fw],
                                        1.0, -1.0, ALU.min, ALU.add)
                nc.vector.tensor_tensor(h2_sb[:, t0c:t0c + shw, :],
                                        e[:, 0:fw], r[:, 0:fw], ALU.add)
                while done:
                    fc1_mms(done.pop(0))
                done.extend(range(t0c, t0c + shw))
            while done:
                fc1_mms(done.pop(0))

            # ---- +bf1/8, 2-way bf16 ReduceScatter over batch blocks -------
            zsb = small.tile([128, 2, H], bf16)
            for bh in range(2):
                nc.vector.tensor_tensor(zsb[:, bh, :], zps[bh][:],
                                        bf1b[:], ALU.add)
            rs_in = dram.tile([NCORE, BPC, H], bf16)
            rs_out = dram.tile([BPC, H], bf16)
            zl = small.tile([BPC, H], bf16)
            nc.sync.dma_start(
                rs_in[:].rearrange("(jh jl) b h -> (jl b) jh h", jh=2),
                zsb[:])
            nc.gpsimd.collective_compute(
                "ReduceScatter", ALU.add,
                replica_groups=[list(range(NCORE))],
                ins=[rs_in.opt()], outs=[rs_out.opt()],
            )
            nc.sync.dma_start(zl[:], rs_out[:])

            # ---- relu, FC2 (+bf2), softmax --------------------------------
            zr = small.tile([BPC, H], f32)
            nc.scalar.activation(zr[:], zl[:], AF.Relu)

            zrT = small.tile([128, HJ, BPC], bf16)
            for hj in range(HJ):
                pt = ps2.tile([128, BPC], f32, tag="ps2")
                nc.tensor.transpose(pt[:], zr[0:BPC, hj * 128:(hj + 1) * 128],
                                    id_sb[:])
                nc.scalar.copy(zrT[:, hj, :], pt[:])

            ps_o = ps1.tile([BPC, N_OUT], f32, tag="ps1")
            for hj in range(HJ):
                nc.tensor.matmul(ps_o[:], zrT[:, hj, :], wf2_sb[:, hj, :],
                                 start=(hj == 0), stop=False)
            nc.tensor.matmul(ps_o[:], ones[0:1, 0:BPC], bf2_sb[:],
                             start=False, stop=True)

            ex = small.tile([BPC, N_OUT], f32)
            sm = small.tile([BPC, 1], f32)
            nc.scalar.activation(ex[:], ps_o[:], AF.Exp, accum_out=sm[:])
            rc = small.tile([BPC, 1], f32)
            nc.vector.reciprocal(rc[:], sm[:])
            ob = small.tile([BPC, N_OUT], f32)
            nc.vector.tensor_scalar(ob[:], ex[:], rc[0:BPC, 0:1], None,
                                    ALU.mult)
            nc.sync.dma_start(out_d[:], ob[:])

    _install_wait_splitter(nc)
    return nc


_NC_CACHE = None


def _get_program():
    global _NC_CACHE
    if _NC_CACHE is None:
        _NC_CACHE = _build_program()
    return _NC_CACHE


# ---------------------------------------------------------------------------
def _prep_inputs(x, edge_row, edge_col, edge_val, W1, b1, W2, b2,
                 Wf1, bf1, Wf2, bf2):
    import ml_dtypes
    f = np.float32
    bf = ml_dtypes.bfloat16
    A = np.zeros((N, N), f)
    np.add.at(A, (np.asarray(edge_row), np.asarray(edge_col)),
              np.asarray(edge_val, f))
    AT = np.ascontiguousarray(A.T)                                  # [m, n]
    at = np.ascontiguousarray(
        AT.reshape(KN, P, N).transpose(1, 0, 2).reshape(P, KN * N)).astype(bf)

    XT = np.ascontiguousarray(np.asarray(x, f)[:, :, 0].T)          # [N, B]
    xt = np.ascontiguousarray(
        XT.reshape(KN, P, B).transpose(1, 0, 2).reshape(P, KN * B)).astype(bf)

    W1 = np.asarray(W1, f); b1 = np.asarray(b1, f)
    W2 = np.asarray(W2, f); b2 = np.asarray(b2, f)
    Wf1 = np.asarray(Wf1, f); bf1 = np.asarray(bf1, f)
    Wf2 = np.asarray(Wf2, f); bf2 = np.asarray(bf2, f)

    # mix weight: lhsT[(s4,c),(s4',c')] = delta(s4,s4') * W2[c,c']
    w2k = np.kron(np.eye(4, dtype=f), W2).astype(bf)                # [128,128]
    b2k = np.tile(b2, 4).reshape(128, 1).astype(f)

    # FC1: core k's K-chunk t = q*SH+sh holds flat rows for node
    # n = 112k + 28q + 4sh + s4, channel c', at partition p = s4*C + c';
    # rows for pad nodes (n >= 784) are zero.
    Wf1_pad = np.zeros((NPAD, C, H), f)
    Wf1_pad[:N] = Wf1.reshape(N, C, H)

    bf18 = (bf1 / NCORE).reshape(1, H).astype(f)
    wf2_l = np.ascontiguousarray(
        Wf2.reshape(HJ, 128, N_OUT).transpose(1, 0, 2).reshape(
            128, HJ * N_OUT)).astype(bf)
    bf2_l = bf2.reshape(1, N_OUT).astype(bf)
    idm = np.eye(BPC, dtype=f)

    in_maps = []
    for k in range(NCORE):
        wb = np.concatenate([W1[0, k * CPC:(k + 1) * CPC],
                             b1[k * CPC:(k + 1) * CPC]]).reshape(1, 2 * CPC)
        # [q, sh, s4, c, H]: chunk (q, sh), partition (s4, c)
        wk = Wf1_pad[k * P:(k + 1) * P].reshape(NG, SH, 4, C, H)
        wf1_l = np.ascontiguousarray(wk.reshape(NS * 128, H)).astype(bf)
        in_maps.append({
            "at": at, "xt": xt, "wf1": wf1_l,
            "wb": np.ascontiguousarray(wb.astype(f)),
            "w2k": w2k, "b2k": b2k,
            "bf18": bf18, "wf2": wf2_l, "bf2": bf2_l, "idm": idm,
        })
    return in_maps


def kernel(x, edge_row, edge_col, edge_val, W1, b1, W2, b2,
           Wf1, bf1, Wf2, bf2, **kw):
    nc = _get_program()
    in_maps = _prep_inputs(x, edge_row, edge_col, edge_val, W1, b1, W2, b2,
                           Wf1, bf1, Wf2, bf2)
    res = run_bass_kernel_spmd(nc, in_maps, list(range(NCORE)), **kw)
    out = np.concatenate([res.results[k]["out"] for k in range(NCORE)], axis=0)
    if kw.get("trace"):
        kernel.last_exec_time_ns = res.exec_time_ns
    return out.astype(np.float32)
